# revision 9
# baseline (speedup 1.0000x reference)
"""GCN classifier (3-layer GCNConv + residual + leaky_relu + global mean pool)
as a Bass/Tile kernel on 8 Trainium2 NeuronCores.

Sharding: nodes are range-partitioned across the 8 cores (6250 each, padded
to 6656); each core owns all edges whose destination lands in its range
(self-loops are materialized as explicit edges, which makes the GCN self-loop
term fall out of the same aggregation). Per layer, each core:
  - dma_gathers the 256B feature rows y[src] (y = x * deg^-1/2, the halo
    exchange tensor) from a DRAM replica filled by an AllGather,
  - segment-sums them into its own nodes with PE indicator matmuls
    (indicator[e, n] = (dst_rel[e] == n) built on DVE via broadcast compare),
  - applies dst-side deg^-1/2, the shared 64x64 weight, bias, residual and
    leaky_relu, and AllGathers the rescaled result for the next layer.
Degrees are computed on device with the same indicator machinery. The final
global-mean-pool partials (feature sums + counts per graph) are computed with
one more indicator matmul; the host sums the 8 partials and divides.
"""

import numpy as np

N = 50000
D = 64
G = 64
L = 3
C = 8
NPC = N // C            # 6250 real nodes per core
TIL = 64                # indicator width / node tile
GRP = 512               # nodes per PSUM group
NPC_PAD = 6656          # 13 * 512 = 52 * 128
NT = NPC_PAD // TIL     # 104 tiles
NGRP = NPC_PAD // GRP   # 13
TPG = GRP // TIL        # 8 tiles per group
HALF = C // 2 * NPC_PAD  # 26624 — first 4 cores' rows
PAD_DST = -1000.0
LRELU_DECOMP = False  # sim-only: bass_interp lacks Lrelu; decompose via Relu


def _host_prep(x, edge_index, batch):
    src = np.asarray(edge_index[0], dtype=np.int64)
    dst = np.asarray(edge_index[1], dtype=np.int64)
    # self loops as explicit edges
    loops = np.arange(N, dtype=np.int64)
    src = np.concatenate([src, loops])
    dst = np.concatenate([dst, loops])

    # padded global row id in the AllGather buffer
    rows = (src // NPC) * NPC_PAD + (src % NPC)
    half = (rows >= HALF).astype(np.int64)
    lrow = rows - half * HALF  # local row within its half, < 26624

    core = dst // NPC
    dloc = dst % NPC
    tile = dloc // TIL
    drel = dloc % TIL

    order = np.lexsort((half, tile, core))
    core_s, tile_s, half_s = core[order], tile[order], half[order]
    lrow_s, drel_s = lrow[order], drel[order]

    key = (core_s * NT + tile_s) * 2 + half_s
    cnt = np.bincount(key, minlength=C * NT * 2).reshape(C, NT, 2)
    chunks = -(-cnt // 128)  # ceil div per (core, tile, half)
    plan = chunks.max(axis=0)          # [NT, 2] — shared across cores
    plan[:, 0] = np.maximum(plan[:, 0], 1)

    starts = np.zeros(C * NT * 2 + 1, np.int64)
    np.cumsum(cnt.reshape(-1), out=starts[1:])

    tot_chunks = int(plan.sum())
    tot_idx = tot_chunks * 128
    gidx = np.zeros((C, tot_idx), np.int16)
    dstrel = np.full((C, tot_chunks * 128), PAD_DST, np.float32)

    batch_chunks = np.zeros((NGRP, 2), np.int64)
    for g in range(NGRP):
        for h in range(2):
            batch_chunks[g, h] = plan[g * TPG:(g + 1) * TPG, h].sum()

    # fill per-core data in batch layout: for g, for h, for t in tiles(g)
    ci = 0
    for g in range(NGRP):
        for h in range(2):
            for tt in range(TPG):
                t = g * TPG + tt
                nch = int(plan[t, h])
                for c in range(C):
                    s = starts[(c * NT + t) * 2 + h]
                    e = starts[(c * NT + t) * 2 + h + 1]
                    n = e - s
                    gidx[c, ci * 128: ci * 128 + n] = lrow_s[s:e]
                    dstrel[c, ci * 128: ci * 128 + n] = drel_s[s:e]
                ci += nch
    assert ci == tot_chunks

    # wrap gather indices per batch block: logical i -> [i % 16, i // 16]
    gidx_w = np.zeros((C, 128, tot_idx // 16), np.int16)
    col = 0
    for g in range(NGRP):
        for h in range(2):
            nb = int(batch_chunks[g, h]) * 128
            blk = gidx[:, col * 16:col * 16 + nb].reshape(C, nb // 16, 16)
            gidx_w[:, :16, col:col + nb // 16] = np.transpose(blk, (0, 2, 1))
            col += nb // 16
    gidx_w = np.tile(gidx_w[:, :16, :], (1, 8, 1))

    dstrel_w = np.ascontiguousarray(
        dstrel.reshape(C, tot_chunks, 128).transpose(0, 2, 1))  # [C,128,TOTC]

    xs, bvs = [], []
    b = np.asarray(batch, dtype=np.int64)
    for c in range(C):
        xp = np.zeros((NPC_PAD, D), np.float32)
        xp[:NPC] = np.asarray(x[c * NPC:(c + 1) * NPC], np.float32)
        xs.append(xp)
        bv = np.full(NPC_PAD, PAD_DST, np.float32)
        bv[:NPC] = b[c * NPC:(c + 1) * NPC].astype(np.float32)
        bvs.append(bv.reshape(NPC_PAD // 128, 128).T.copy())  # [128, 52]
    return xs, bvs, gidx_w, dstrel_w, batch_chunks, plan, tot_chunks


_BUILD_CACHE = {}


def _build(batch_chunks, plan, tot_chunks):
    import concourse.bacc as bacc
    import concourse.tile as tile
    import concourse.mybir as mybir

    f32 = mybir.dt.float32
    TOTC = tot_chunks
    MAXCH = int(batch_chunks.max())
    AF = mybir.ActivationFunctionType
    ALU = mybir.AluOpType

    nc = bacc.Bacc("TRN2", target_bir_lowering=False, debug=False, num_devices=C)

    iota_c = nc.inline_tensor(
        np.tile(np.arange(TIL, dtype=np.float32)[None, :], (128, 1)), name="iota_c")
    id_c = nc.inline_tensor(np.eye(128, dtype=np.float32), name="id_c")
    ones_col_c = nc.inline_tensor(np.ones((128, 1), np.float32), name="ones_col_c")
    ones_row_c = nc.inline_tensor(np.ones((1, 512), np.float32), name="ones_row_c")

    # chunk/idx col base per (g, h) batch
    cbase = np.zeros((NGRP, 2), np.int64)
    acc = 0
    for g in range(NGRP):
        for h in range(2):
            cbase[g, h] = acc
            acc += int(batch_chunks[g, h])
    # chunk offset of tile tt within batch (g, h)
    toff = np.zeros((NGRP, 2, TPG), np.int64)
    for g in range(NGRP):
        for h in range(2):
            o = 0
            for tt in range(TPG):
                toff[g, h, tt] = o
                o += int(plan[g * TPG + tt, h])

    with tile.TileContext(nc) as tc:
        with tc.tile_pool(name="dram", bufs=1, space="DRAM") as dram, \
             tc.tile_pool(name="per", bufs=1) as per, \
             tc.tile_pool(name="wrk", bufs=3) as wrk, \
             tc.tile_pool(name="sml", bufs=2) as sml, \
             tc.tile_pool(name="ps", bufs=2, space="PSUM") as ps:

            x_own = dram.tile([NPC_PAD, D], f32, kind="ExternalInput", name="x_own", uniquify=False)
            gidx_t = dram.tile([128, TOTC * 8], mybir.dt.int16, kind="ExternalInput", name="gidx", uniquify=False)
            dstrel_t = dram.tile([128, TOTC], f32, kind="ExternalInput", name="dstrel", uniquify=False)
            batchv_t = dram.tile([128, NPC_PAD // 128], f32, kind="ExternalInput", name="batchv", uniquify=False)
            Ws_t = dram.tile([L, D, D], f32, kind="ExternalInput", name="Ws", uniquify=False)
            bs_t = dram.tile([L, D], f32, kind="ExternalInput", name="bs", uniquify=False)
            out_t = dram.tile([D + 1, G], f32, kind="ExternalOutput", name="out_partial", uniquify=False)

            y_shard = [dram.tile([NPC_PAD, D], f32, kind="Internal", name=f"y_shard{l}")
                       for l in range(L)]
            y_full = [dram.tile([C * NPC_PAD, D], f32, kind="Internal",
                                addr_space="Shared", name=f"y_full{l}")
                      for l in range(L)]
            dinv_dram = dram.tile([NPC_PAD // 128, 128], f32, kind="Internal", name="dinv_dram")

            # ---- persistent SBUF ----
            iota_sb = per.tile([128, TIL], f32)
            nc.sync.dma_start(iota_sb[:], iota_c[:])
            id_sb = per.tile([128, 128], f32)
            nc.sync.dma_start(id_sb[:], id_c[:])
            onec_sb = per.tile([128, 1], f32)
            nc.sync.dma_start(onec_sb[:], ones_col_c[:])
            oner_sb = per.tile([1, 512], f32)
            nc.sync.dma_start(oner_sb[:], ones_row_c[:])
            dstrel_sb = per.tile([128, TOTC], f32)
            nc.sync.dma_start(dstrel_sb[:], dstrel_t[:])
            batchv_sb = per.tile([128, NPC_PAD // 128], f32)
            nc.sync.dma_start(batchv_sb[:], batchv_t[:])
            Ws_sb = per.tile([D, L, D], f32)
            nc.sync.dma_start(Ws_sb[:], Ws_t[:].rearrange("l k m -> k l m"))
            bs_sb = per.tile([1, L, D], f32)
            nc.sync.dma_start(bs_sb[:], bs_t[:].rearrange("l m -> () l m"))

            y_nm = per.tile([128, NPC_PAD // 128, D], f32)  # node-major staging
            nc.sync.dma_start(y_nm[:], x_own[:].rearrange("(g p) f -> p g f", p=128))
            x3_aug = per.tile([128, NPC_PAD // 128, D + 1], f32)
            nc.vector.memset(x3_aug[:, :, D:D + 1], 1.0)
            xT = per.tile([D, NPC_PAD], f32)          # current x, feature-major
            dinv_row = per.tile([1, NPC_PAD], f32)
            dinv_bc = per.tile([D, NPC_PAD], f32)     # dinv broadcast to 64 partitions
            dinv_nm = per.tile([128, NPC_PAD // 128], f32)

            def build_ind(g, h):
                nbc = int(batch_chunks[g, h])
                cb = int(cbase[g, h])
                ind = wrk.tile([128, MAXCH, TIL], f32, tag="ind")
                nc.vector.tensor_tensor(
                    out=ind[:, 0:nbc, :],
                    in0=iota_sb[:, None, :].to_broadcast([128, nbc, TIL]),
                    in1=dstrel_sb[:, cb:cb + nbc, None].to_broadcast([128, nbc, TIL]),
                    op=ALU.is_equal)
                return ind

            def flags(g, tt, h, j):
                t = g * TPG + tt
                first = h == 0 and j == 0
                last = ((h == 1 and j == plan[t, 1] - 1)
                        or (h == 0 and plan[t, 1] == 0 and j == plan[t, 0] - 1))
                return bool(first), bool(last)

            # ================= degree pass =================
            for g in range(NGRP):
                deg_ps = ps.tile([1, 512], f32, space="PSUM", tag="tp")
                inds = [build_ind(g, 0), build_ind(g, 1)]
                for tt in range(TPG):
                    t = g * TPG + tt
                    for h in range(2):
                        for j in range(int(plan[t, h])):
                            first, last = flags(g, tt, h, j)
                            jj = int(toff[g, h, tt]) + j
                            nc.tensor.matmul(
                                out=deg_ps[0:1, tt * TIL:(tt + 1) * TIL],
                                lhsT=onec_sb[:, 0:1], rhs=inds[h][:, jj, :],
                                start=first, stop=last)
                dmax = sml.tile([1, 512], f32, tag="dr")
                nc.vector.tensor_scalar_max(out=dmax[:], in0=deg_ps[:], scalar1=1.0)
                dsq = sml.tile([1, 512], f32, tag="dr2")
                nc.scalar.activation(out=dsq[:], in_=dmax[:], func=AF.Sqrt)
                nc.vector.reciprocal(out=dinv_row[:, g * 512:(g + 1) * 512], in_=dsq[:])

            # dinv broadcast tiles (feature-major) and node-major copy
            for g in range(NGRP):
                bc_ps = ps.tile([D, 512], f32, space="PSUM", tag="tp")
                nc.tensor.matmul(out=bc_ps[:], lhsT=oner_sb[0:1, 0:D],
                                 rhs=dinv_row[:, g * 512:(g + 1) * 512],
                                 start=True, stop=True)
                nc.vector.tensor_copy(out=dinv_bc[:, g * 512:(g + 1) * 512], in_=bc_ps[:])
            nc.sync.dma_start(dinv_dram[:].rearrange("g p -> () (g p)"), dinv_row[:])
            nc.sync.dma_start(dinv_nm[:], dinv_dram[:].rearrange("g p -> p g"))

            # y0 = x * dinv (node-major, in place), export + AllGather
            nc.vector.tensor_tensor(
                out=y_nm[:], in0=y_nm[:],
                in1=dinv_nm[:, :, None].to_broadcast([128, NPC_PAD // 128, D]),
                op=ALU.mult)
            nc.sync.dma_start(y_shard[0][:].rearrange("(g p) f -> p g f", p=128), y_nm[:])
            nc.gpsimd.collective_compute(
                "AllGather", ALU.bypass, replica_groups=[list(range(C))],
                ins=[y_shard[0][:]], outs=[y_full[0][:]])

            # ================= layers =================
            for l in range(L):
                for g in range(NGRP):
                    agg_ps = ps.tile([D, 512], f32, space="PSUM", tag="agg")
                    msgs = []
                    for h in range(2):
                        nbc = int(batch_chunks[g, h])
                        cb = int(cbase[g, h])
                        nb = nbc * 128
                        gi = wrk.tile([128, MAXCH * 8], mybir.dt.int16, tag="gi")
                        nc.sync.dma_start(gi[:, 0:nb // 16],
                                          gidx_t[:, cb * 8:cb * 8 + nb // 16])
                        m = wrk.tile([128, MAXCH, D], f32, tag="msgs")
                        src_ap = y_full[l][HALF:, :] if h else y_full[l][0:HALF, :]
                        nc.gpsimd.dma_gather(
                            m[:, 0:nbc, :], src_ap, gi[:, 0:nb // 16], nb, nb, D,
                            single_packet=False)
                        msgs.append(m)
                    inds = [build_ind(g, 0), build_ind(g, 1)]
                    for tt in range(TPG):
                        t = g * TPG + tt
                        for h in range(2):
                            for j in range(int(plan[t, h])):
                                first, last = flags(g, tt, h, j)
                                jj = int(toff[g, h, tt]) + j
                                nc.tensor.matmul(
                                    out=agg_ps[:, tt * TIL:(tt + 1) * TIL],
                                    lhsT=msgs[h][:, jj, :], rhs=inds[h][:, jj, :],
                                    start=first, stop=last)
                    # epilogue for this 512-node group
                    sl = slice(g * 512, (g + 1) * 512)
                    rhs_sb = sml.tile([D, 512], f32, tag="rhs")
                    nc.vector.tensor_tensor(out=rhs_sb[:], in0=agg_ps[:],
                                            in1=dinv_bc[:, sl], op=ALU.mult)
                    tr_ps = ps.tile([D, 512], f32, space="PSUM", tag="tr")
                    if l > 0:
                        nc.tensor.matmul(out=tr_ps[:], lhsT=id_sb[0:D, 0:D],
                                         rhs=xT[:, sl], start=True, stop=False)
                    nc.tensor.matmul(out=tr_ps[:], lhsT=Ws_sb[:, l, :], rhs=rhs_sb[:],
                                     start=(l == 0), stop=False)
                    nc.tensor.matmul(out=tr_ps[:], lhsT=bs_sb[:, l, :], rhs=oner_sb[:],
                                     start=False, stop=True)
                    if LRELU_DECOMP:
                        r_sb = sml.tile([D, 512], f32, tag="lr1")
                        nc.scalar.activation(out=r_sb[:], in_=tr_ps[:], func=AF.Relu)
                        t_sb = sml.tile([D, 512], f32, tag="lr2")
                        nc.scalar.activation(out=t_sb[:], in_=tr_ps[:],
                                             func=AF.Copy, scale=0.01)
                        nc.vector.scalar_tensor_tensor(
                            out=xT[:, sl], in0=r_sb[:], scalar=0.99, in1=t_sb[:],
                            op0=ALU.mult, op1=ALU.add)
                    else:
                        nc.scalar.activation(out=xT[:, sl], in_=tr_ps[:],
                                             func=AF.Lrelu, alpha=0.01)
                    tp_ps = ps.tile([128, 256], f32, space="PSUM", tag="tp")
                    if l < L - 1:
                        yT = sml.tile([D, 512], f32, tag="yT")
                        nc.vector.tensor_tensor(out=yT[:], in0=xT[:, sl],
                                                in1=dinv_bc[:, sl], op=ALU.mult)
                        for k in range(4):
                            nc.tensor.transpose(out=tp_ps[:, k * D:(k + 1) * D],
                                                in_=yT[:, k * 128:(k + 1) * 128],
                                                identity=id_sb[0:D, 0:D])
                        nc.vector.tensor_copy(
                            out=y_nm[:, g * 4:(g + 1) * 4, :],
                            in_=tp_ps[:].rearrange("p (g f) -> p g f", f=D))
                    else:
                        for k in range(4):
                            nc.tensor.transpose(out=tp_ps[:, k * D:(k + 1) * D],
                                                in_=xT[:, g * 512 + k * 128: g * 512 + (k + 1) * 128],
                                                identity=id_sb[0:D, 0:D])
                        nc.vector.tensor_copy(
                            out=x3_aug[:, g * 4:(g + 1) * 4, 0:D],
                            in_=tp_ps[:].rearrange("p (g f) -> p g f", f=D))
                if l < L - 1:
                    nc.sync.dma_start(
                        y_shard[l + 1][:].rearrange("(g p) f -> p g f", p=128), y_nm[:])
                    nc.gpsimd.collective_compute(
                        "AllGather", ALU.bypass, replica_groups=[list(range(C))],
                        ins=[y_shard[l + 1][:]], outs=[y_full[l + 1][:]])

            # ================= pooling =================
            NCG = NPC_PAD // 128  # 52
            pind = wrk.tile([128, NCG, G], f32, tag="ind")
            nc.vector.tensor_tensor(
                out=pind[:],
                in0=iota_sb[:, None, :].to_broadcast([128, NCG, G]),
                in1=batchv_sb[:, :, None].to_broadcast([128, NCG, G]),
                op=ALU.is_equal)
            pool_ps = ps.tile([D + 1, G], f32, space="PSUM", tag="tr")
            for t in range(NCG):
                nc.tensor.matmul(out=pool_ps[:], lhsT=x3_aug[:, t, :], rhs=pind[:, t, :],
                                 start=(t == 0), stop=(t == NCG - 1))
            pool_sb = sml.tile([D + 1, G], f32, tag="dr")
            nc.vector.tensor_copy(out=pool_sb[:], in_=pool_ps[:])
            nc.sync.dma_start(out_t[:], pool_sb[:])

    nc.compile()
    return nc


def kernel(x, edge_index, batch, Ws, bs):
    from concourse.bass_utils import run_bass_kernel_spmd

    x = np.asarray(x, np.float32)
    Ws_np = np.asarray(Ws, np.float32)
    bs_np = np.asarray(bs, np.float32)

    xs, bvs, gidx_w, dstrel_w, batch_chunks, plan, tot_chunks = _host_prep(
        x, edge_index, batch)

    key = (batch_chunks.tobytes(), plan.tobytes())
    if key not in _BUILD_CACHE:
        _BUILD_CACHE[key] = _build(batch_chunks, plan, tot_chunks)
    nc = _BUILD_CACHE[key]

    in_maps = []
    for c in range(C):
        in_maps.append({
            "x_own": xs[c],
            "gidx": np.ascontiguousarray(gidx_w[c]),
            "dstrel": np.ascontiguousarray(dstrel_w[c]),
            "batchv": np.ascontiguousarray(bvs[c]),
            "Ws": Ws_np,
            "bs": bs_np,
        })
    res = run_bass_kernel_spmd(nc, in_maps, core_ids=list(range(C)))

    total = np.zeros((D + 1, G), np.float64)
    for c in range(C):
        total += res.results[c]["out_partial"].astype(np.float64)
    sums = total[:D]                    # [feat, graph]
    counts = np.maximum(total[D], 1.0)  # [graph]
    pooled = (sums / counts[None, :]).T.astype(np.float32)
    return pooled


# revision 11
# speedup vs baseline: 83.9690x; 83.9690x over previous
"""GCN classifier (3-layer GCNConv + residual + leaky_relu + global mean pool)
as a Bass/Tile kernel on 8 Trainium2 NeuronCores.

Sharding: nodes are range-partitioned across the 8 cores (6250 each, padded
to 6656); each core owns all edges whose destination lands in its range
(self-loops are materialized as explicit edges, which makes the GCN self-loop
term fall out of the same aggregation). Per layer, each core:
  - dma_gathers the 256B feature rows y[src] (y = x * deg^-1/2, the halo
    exchange tensor) from a DRAM replica filled by an AllGather,
  - segment-sums them into its own nodes with PE indicator matmuls
    (indicator[e, n] = (dst_rel[e] == n) built on DVE via broadcast compare),
  - applies dst-side deg^-1/2, the shared 64x64 weight, bias, residual and
    leaky_relu, and AllGathers the rescaled result for the next layer.
Degrees are computed on device with the same indicator machinery. The final
global-mean-pool partials (feature sums + counts per graph) are computed with
one more indicator matmul; the host sums the 8 partials and divides.
"""

import numpy as np

N = 50000
D = 64
G = 64
L = 3
C = 8
NPC = N // C            # 6250 real nodes per core
TIL = 64                # indicator width / node tile
GRP = 512               # nodes per PSUM group
NPC_PAD = 6656          # 13 * 512 = 52 * 128
NT = NPC_PAD // TIL     # 104 tiles
NGRP = NPC_PAD // GRP   # 13
TPG = GRP // TIL        # 8 tiles per group
HALF = C // 2 * NPC_PAD  # 26624 — first 4 cores' rows
PAD_DST = -1000.0
LRELU_DECOMP = False  # sim-only: bass_interp lacks Lrelu; decompose via Relu
TRACE = False         # test-only: capture NTFF profile, report exec_time_ns
LAST_RESULT = None    # test-only: BassKernelResults of the last run


def _host_prep(x, edge_index, batch):
    src = np.asarray(edge_index[0], dtype=np.int64)
    dst = np.asarray(edge_index[1], dtype=np.int64)
    # self loops as explicit edges
    loops = np.arange(N, dtype=np.int64)
    src = np.concatenate([src, loops])
    dst = np.concatenate([dst, loops])

    # padded global row id in the AllGather buffer
    rows = (src // NPC) * NPC_PAD + (src % NPC)
    half = (rows >= HALF).astype(np.int64)
    lrow = rows - half * HALF  # local row within its half, < 26624

    core = dst // NPC
    dloc = dst % NPC
    tile = dloc // TIL
    drel = dloc % TIL

    order = np.lexsort((half, tile, core))
    core_s, tile_s, half_s = core[order], tile[order], half[order]
    lrow_s, drel_s = lrow[order], drel[order]

    key = (core_s * NT + tile_s) * 2 + half_s
    cnt = np.bincount(key, minlength=C * NT * 2).reshape(C, NT, 2)
    chunks = -(-cnt // 128)  # ceil div per (core, tile, half)
    plan = chunks.max(axis=0)          # [NT, 2] — shared across cores
    plan[:, 0] = np.maximum(plan[:, 0], 1)

    starts = np.zeros(C * NT * 2 + 1, np.int64)
    np.cumsum(cnt.reshape(-1), out=starts[1:])

    tot_chunks = int(plan.sum())
    tot_idx = tot_chunks * 128
    gidx = np.zeros((C, tot_idx), np.int16)
    dstrel = np.full((C, tot_chunks * 128), PAD_DST, np.float32)

    batch_chunks = np.zeros((NGRP, 2), np.int64)
    for g in range(NGRP):
        for h in range(2):
            batch_chunks[g, h] = plan[g * TPG:(g + 1) * TPG, h].sum()

    # fill per-core data in batch layout: for g, for h, for t in tiles(g)
    ci = 0
    for g in range(NGRP):
        for h in range(2):
            for tt in range(TPG):
                t = g * TPG + tt
                nch = int(plan[t, h])
                for c in range(C):
                    s = starts[(c * NT + t) * 2 + h]
                    e = starts[(c * NT + t) * 2 + h + 1]
                    n = e - s
                    gidx[c, ci * 128: ci * 128 + n] = lrow_s[s:e]
                    dstrel[c, ci * 128: ci * 128 + n] = drel_s[s:e]
                ci += nch
    assert ci == tot_chunks

    # wrap gather indices per batch block: logical i -> [i % 16, i // 16]
    gidx_w = np.zeros((C, 128, tot_idx // 16), np.int16)
    col = 0
    for g in range(NGRP):
        for h in range(2):
            nb = int(batch_chunks[g, h]) * 128
            blk = gidx[:, col * 16:col * 16 + nb].reshape(C, nb // 16, 16)
            gidx_w[:, :16, col:col + nb // 16] = np.transpose(blk, (0, 2, 1))
            col += nb // 16
    gidx_w = np.tile(gidx_w[:, :16, :], (1, 8, 1))

    dstrel_w = np.ascontiguousarray(
        dstrel.reshape(C, tot_chunks, 128).transpose(0, 2, 1))  # [C,128,TOTC]

    xs, bvs = [], []
    b = np.asarray(batch, dtype=np.int64)
    for c in range(C):
        xp = np.zeros((NPC_PAD, D), np.float32)
        xp[:NPC] = np.asarray(x[c * NPC:(c + 1) * NPC], np.float32)
        xs.append(xp)
        bv = np.full(NPC_PAD, PAD_DST, np.float32)
        bv[:NPC] = b[c * NPC:(c + 1) * NPC].astype(np.float32)
        bvs.append(bv.reshape(NPC_PAD // 128, 128).T.copy())  # [128, 52]
    return xs, bvs, gidx_w, dstrel_w, batch_chunks, plan, tot_chunks


_BUILD_CACHE = {}


def _build(batch_chunks, plan, tot_chunks):
    import concourse.bacc as bacc
    import concourse.tile as tile
    import concourse.mybir as mybir

    f32 = mybir.dt.float32
    TOTC = tot_chunks
    MAXCH = int(batch_chunks.max())
    AF = mybir.ActivationFunctionType
    ALU = mybir.AluOpType

    nc = bacc.Bacc("TRN2", target_bir_lowering=False, debug=False, num_devices=C)

    iota_c = nc.inline_tensor(
        np.tile(np.arange(TIL, dtype=np.float32)[None, :], (128, 1)), name="iota_c")
    id_c = nc.inline_tensor(np.eye(128, dtype=np.float32), name="id_c")
    ones_col_c = nc.inline_tensor(np.ones((128, 1), np.float32), name="ones_col_c")
    ones_row_c = nc.inline_tensor(np.ones((1, 512), np.float32), name="ones_row_c")

    # chunk/idx col base per (g, h) batch
    cbase = np.zeros((NGRP, 2), np.int64)
    acc = 0
    for g in range(NGRP):
        for h in range(2):
            cbase[g, h] = acc
            acc += int(batch_chunks[g, h])
    # chunk offset of tile tt within batch (g, h)
    toff = np.zeros((NGRP, 2, TPG), np.int64)
    for g in range(NGRP):
        for h in range(2):
            o = 0
            for tt in range(TPG):
                toff[g, h, tt] = o
                o += int(plan[g * TPG + tt, h])

    with tile.TileContext(nc) as tc:
        with tc.tile_pool(name="dram", bufs=1, space="DRAM") as dram, \
             tc.tile_pool(name="per", bufs=1) as per, \
             tc.tile_pool(name="wrk", bufs=3) as wrk, \
             tc.tile_pool(name="sml", bufs=2) as sml, \
             tc.tile_pool(name="ps", bufs=2, space="PSUM") as ps:

            x_own = dram.tile([NPC_PAD, D], f32, kind="ExternalInput", name="x_own", uniquify=False)
            gidx_t = dram.tile([128, TOTC * 8], mybir.dt.int16, kind="ExternalInput", name="gidx", uniquify=False)
            dstrel_t = dram.tile([128, TOTC], f32, kind="ExternalInput", name="dstrel", uniquify=False)
            batchv_t = dram.tile([128, NPC_PAD // 128], f32, kind="ExternalInput", name="batchv", uniquify=False)
            Ws_t = dram.tile([L, D, D], f32, kind="ExternalInput", name="Ws", uniquify=False)
            bs_t = dram.tile([L, D], f32, kind="ExternalInput", name="bs", uniquify=False)
            out_t = dram.tile([D + 1, G], f32, kind="ExternalOutput", name="out_partial", uniquify=False)

            y_shard = [dram.tile([NPC_PAD, D], f32, kind="Internal", name=f"y_shard{l}")
                       for l in range(L)]
            y_full = [dram.tile([C * NPC_PAD, D], f32, kind="Internal",
                                addr_space="Shared", name=f"y_full{l}")
                      for l in range(L)]
            dinv_dram = dram.tile([NPC_PAD // 128, 128], f32, kind="Internal", name="dinv_dram")

            # ---- persistent SBUF ----
            iota_sb = per.tile([128, TIL], f32)
            nc.sync.dma_start(iota_sb[:], iota_c[:])
            id_sb = per.tile([128, 128], f32)
            nc.sync.dma_start(id_sb[:], id_c[:])
            onec_sb = per.tile([128, 1], f32)
            nc.sync.dma_start(onec_sb[:], ones_col_c[:])
            oner_sb = per.tile([1, 512], f32)
            nc.sync.dma_start(oner_sb[:], ones_row_c[:])
            dstrel_sb = per.tile([128, TOTC], f32)
            nc.sync.dma_start(dstrel_sb[:], dstrel_t[:])
            batchv_sb = per.tile([128, NPC_PAD // 128], f32)
            nc.sync.dma_start(batchv_sb[:], batchv_t[:])
            Ws_sb = per.tile([D, L, D], f32)
            nc.sync.dma_start(Ws_sb[:], Ws_t[:].rearrange("l k m -> k l m"))
            bs_sb = per.tile([1, L, D], f32)
            nc.sync.dma_start(bs_sb[:], bs_t[:].rearrange("l m -> () l m"))

            y_nm = per.tile([128, NPC_PAD // 128, D], f32)  # node-major staging
            nc.sync.dma_start(y_nm[:], x_own[:].rearrange("(g p) f -> p g f", p=128))
            x3_aug = per.tile([128, NPC_PAD // 128, D + 1], f32)
            nc.vector.memset(x3_aug[:, :, D:D + 1], 1.0)
            xT = per.tile([D, NPC_PAD], f32)          # current x, feature-major
            dinv_row = per.tile([1, NPC_PAD], f32)
            dinv_bc = per.tile([D, NPC_PAD], f32)     # dinv broadcast to 64 partitions
            dinv_nm = per.tile([128, NPC_PAD // 128], f32)

            def build_ind(g, h):
                nbc = int(batch_chunks[g, h])
                cb = int(cbase[g, h])
                ind = wrk.tile([128, MAXCH, TIL], f32, tag="ind")
                nc.vector.tensor_tensor(
                    out=ind[:, 0:nbc, :],
                    in0=iota_sb[:, None, :].to_broadcast([128, nbc, TIL]),
                    in1=dstrel_sb[:, cb:cb + nbc, None].to_broadcast([128, nbc, TIL]),
                    op=ALU.is_equal)
                return ind

            def flags(g, tt, h, j):
                t = g * TPG + tt
                first = h == 0 and j == 0
                last = ((h == 1 and j == plan[t, 1] - 1)
                        or (h == 0 and plan[t, 1] == 0 and j == plan[t, 0] - 1))
                return bool(first), bool(last)

            # ================= degree pass =================
            for g in range(NGRP):
                deg_ps = ps.tile([1, 512], f32, space="PSUM", tag="tp")
                inds = [build_ind(g, 0), build_ind(g, 1)]
                for tt in range(TPG):
                    t = g * TPG + tt
                    for h in range(2):
                        for j in range(int(plan[t, h])):
                            first, last = flags(g, tt, h, j)
                            jj = int(toff[g, h, tt]) + j
                            nc.tensor.matmul(
                                out=deg_ps[0:1, tt * TIL:(tt + 1) * TIL],
                                lhsT=onec_sb[:, 0:1], rhs=inds[h][:, jj, :],
                                start=first, stop=last)
                dmax = sml.tile([1, 512], f32, tag="dr")
                nc.vector.tensor_scalar_max(out=dmax[:], in0=deg_ps[:], scalar1=1.0)
                dsq = sml.tile([1, 512], f32, tag="dr2")
                nc.scalar.activation(out=dsq[:], in_=dmax[:], func=AF.Sqrt)
                nc.vector.reciprocal(out=dinv_row[:, g * 512:(g + 1) * 512], in_=dsq[:])

            # dinv broadcast tiles (feature-major) and node-major copy
            for g in range(NGRP):
                bc_ps = ps.tile([D, 512], f32, space="PSUM", tag="tp")
                nc.tensor.matmul(out=bc_ps[:], lhsT=oner_sb[0:1, 0:D],
                                 rhs=dinv_row[:, g * 512:(g + 1) * 512],
                                 start=True, stop=True)
                nc.vector.tensor_copy(out=dinv_bc[:, g * 512:(g + 1) * 512], in_=bc_ps[:])
            nc.sync.dma_start(dinv_dram[:].rearrange("g p -> () (g p)"), dinv_row[:])
            nc.sync.dma_start(dinv_nm[:], dinv_dram[:].rearrange("g p -> p g"))

            # y0 = x * dinv (node-major, in place), export + AllGather
            nc.vector.tensor_tensor(
                out=y_nm[:], in0=y_nm[:],
                in1=dinv_nm[:, :, None].to_broadcast([128, NPC_PAD // 128, D]),
                op=ALU.mult)
            nc.sync.dma_start(y_shard[0][:].rearrange("(g p) f -> p g f", p=128), y_nm[:])
            nc.gpsimd.collective_compute(
                "AllGather", ALU.bypass, replica_groups=[list(range(C))],
                ins=[y_shard[0][:]], outs=[y_full[0][:]])

            # ================= layers =================
            for l in range(L):
                for g in range(NGRP):
                    agg_ps = ps.tile([D, 512], f32, space="PSUM", tag="agg")
                    msgs = []
                    for h in range(2):
                        nbc = int(batch_chunks[g, h])
                        cb = int(cbase[g, h])
                        nb = nbc * 128
                        gi = wrk.tile([128, MAXCH * 8], mybir.dt.int16, tag="gi")
                        nc.sync.dma_start(gi[:, 0:nb // 16],
                                          gidx_t[:, cb * 8:cb * 8 + nb // 16])
                        m = wrk.tile([128, MAXCH, D], f32, tag="msgs")
                        src_ap = y_full[l][HALF:, :] if h else y_full[l][0:HALF, :]
                        nc.gpsimd.dma_gather(
                            m[:, 0:nbc, :], src_ap, gi[:, 0:nb // 16], nb, nb, D,
                            single_packet=False)
                        msgs.append(m)
                    inds = [build_ind(g, 0), build_ind(g, 1)]
                    for tt in range(TPG):
                        t = g * TPG + tt
                        for h in range(2):
                            for j in range(int(plan[t, h])):
                                first, last = flags(g, tt, h, j)
                                jj = int(toff[g, h, tt]) + j
                                nc.tensor.matmul(
                                    out=agg_ps[:, tt * TIL:(tt + 1) * TIL],
                                    lhsT=msgs[h][:, jj, :], rhs=inds[h][:, jj, :],
                                    start=first, stop=last)
                    # epilogue for this 512-node group
                    sl = slice(g * 512, (g + 1) * 512)
                    rhs_sb = sml.tile([D, 512], f32, tag="rhs")
                    nc.vector.tensor_tensor(out=rhs_sb[:], in0=agg_ps[:],
                                            in1=dinv_bc[:, sl], op=ALU.mult)
                    tr_ps = ps.tile([D, 512], f32, space="PSUM", tag="tr")
                    if l > 0:
                        nc.tensor.matmul(out=tr_ps[:], lhsT=id_sb[0:D, 0:D],
                                         rhs=xT[:, sl], start=True, stop=False)
                    nc.tensor.matmul(out=tr_ps[:], lhsT=Ws_sb[:, l, :], rhs=rhs_sb[:],
                                     start=(l == 0), stop=False)
                    nc.tensor.matmul(out=tr_ps[:], lhsT=bs_sb[:, l, :], rhs=oner_sb[:],
                                     start=False, stop=True)
                    if LRELU_DECOMP:
                        r_sb = sml.tile([D, 512], f32, tag="lr1")
                        nc.scalar.activation(out=r_sb[:], in_=tr_ps[:], func=AF.Relu)
                        t_sb = sml.tile([D, 512], f32, tag="lr2")
                        nc.scalar.activation(out=t_sb[:], in_=tr_ps[:],
                                             func=AF.Copy, scale=0.01)
                        nc.vector.scalar_tensor_tensor(
                            out=xT[:, sl], in0=r_sb[:], scalar=0.99, in1=t_sb[:],
                            op0=ALU.mult, op1=ALU.add)
                    else:
                        nc.scalar.activation(out=xT[:, sl], in_=tr_ps[:],
                                             func=AF.Lrelu, alpha=0.01)
                    tp_ps = ps.tile([128, 256], f32, space="PSUM", tag="tp")
                    if l < L - 1:
                        yT = sml.tile([D, 512], f32, tag="yT")
                        nc.vector.tensor_tensor(out=yT[:], in0=xT[:, sl],
                                                in1=dinv_bc[:, sl], op=ALU.mult)
                        for k in range(4):
                            nc.tensor.transpose(out=tp_ps[:, k * D:(k + 1) * D],
                                                in_=yT[:, k * 128:(k + 1) * 128],
                                                identity=id_sb[0:D, 0:D])
                        nc.vector.tensor_copy(
                            out=y_nm[:, g * 4:(g + 1) * 4, :],
                            in_=tp_ps[:].rearrange("p (g f) -> p g f", f=D))
                    else:
                        for k in range(4):
                            nc.tensor.transpose(out=tp_ps[:, k * D:(k + 1) * D],
                                                in_=xT[:, g * 512 + k * 128: g * 512 + (k + 1) * 128],
                                                identity=id_sb[0:D, 0:D])
                        nc.vector.tensor_copy(
                            out=x3_aug[:, g * 4:(g + 1) * 4, 0:D],
                            in_=tp_ps[:].rearrange("p (g f) -> p g f", f=D))
                if l < L - 1:
                    nc.sync.dma_start(
                        y_shard[l + 1][:].rearrange("(g p) f -> p g f", p=128), y_nm[:])
                    nc.gpsimd.collective_compute(
                        "AllGather", ALU.bypass, replica_groups=[list(range(C))],
                        ins=[y_shard[l + 1][:]], outs=[y_full[l + 1][:]])

            # ================= pooling =================
            NCG = NPC_PAD // 128  # 52
            pind = wrk.tile([128, NCG, G], f32, tag="ind")
            nc.vector.tensor_tensor(
                out=pind[:],
                in0=iota_sb[:, None, :].to_broadcast([128, NCG, G]),
                in1=batchv_sb[:, :, None].to_broadcast([128, NCG, G]),
                op=ALU.is_equal)
            pool_ps = ps.tile([D + 1, G], f32, space="PSUM", tag="tr")
            for t in range(NCG):
                nc.tensor.matmul(out=pool_ps[:], lhsT=x3_aug[:, t, :], rhs=pind[:, t, :],
                                 start=(t == 0), stop=(t == NCG - 1))
            pool_sb = sml.tile([D + 1, G], f32, tag="dr")
            nc.vector.tensor_copy(out=pool_sb[:], in_=pool_ps[:])
            nc.sync.dma_start(out_t[:], pool_sb[:])

    nc.compile()
    return nc


def kernel(x, edge_index, batch, Ws, bs):
    from concourse.bass_utils import run_bass_kernel_spmd

    x = np.asarray(x, np.float32)
    Ws_np = np.asarray(Ws, np.float32)
    bs_np = np.asarray(bs, np.float32)

    xs, bvs, gidx_w, dstrel_w, batch_chunks, plan, tot_chunks = _host_prep(
        x, edge_index, batch)

    key = (batch_chunks.tobytes(), plan.tobytes())
    if key not in _BUILD_CACHE:
        _BUILD_CACHE[key] = _build(batch_chunks, plan, tot_chunks)
    nc = _BUILD_CACHE[key]

    in_maps = []
    for c in range(C):
        in_maps.append({
            "x_own": xs[c],
            "gidx": np.ascontiguousarray(gidx_w[c]),
            "dstrel": np.ascontiguousarray(dstrel_w[c]),
            "batchv": np.ascontiguousarray(bvs[c]),
            "Ws": Ws_np,
            "bs": bs_np,
        })
    res = run_bass_kernel_spmd(nc, in_maps, core_ids=list(range(C)), trace=TRACE)
    global LAST_RESULT
    LAST_RESULT = res

    total = np.zeros((D + 1, G), np.float64)
    for c in range(C):
        total += res.results[c]["out_partial"].astype(np.float64)
    sums = total[:D]                    # [feat, graph]
    counts = np.maximum(total[D], 1.0)  # [graph]
    pooled = (sums / counts[None, :]).T.astype(np.float32)
    return pooled
